# revision 19
# baseline (speedup 1.0000x reference)
"""MultiHeadAttention Trainium2 kernel.

Full shapes: B=4, T=2048, D=1024, H=16, HD=64.
Sharding over 8 cores: core c -> batch b=c//2, head-group g=c%2 (8 heads,
feature columns g*512:(g+1)*512 of the projection space).

Per-core program (single SPMD Bass program, different input shards):
  - Prologue: only K/Q projections of chunk 0 (the two gates of the first
    exp). Everything else (V chunk 0, K/V chunks 1-3, Q chunks 1-3, output
    projections) is emitted as deadline-driven *fillers* inside the
    attention stream, so ScalarE (the bottleneck: ~285us of exp work/core)
    starts at ~34us and TensorE idle gaps absorb the projection work.
  - V stored as vp4[tok, tkt, head, 65]: 64 projected features plus a ones
    column, so each PV matmul (M=65) accumulates the softmax denominator in
    PSUM row 64 for free -- no separate ones-matmuls.
  - Attention: flat software-pipelined batch stream over (chunk, pair, j):
    scores^T [tk, tq] with two heads row-packed at PE rows 0/64; one
    ScalarE Exp per [128, 2, 512] PSUM tile (double-buffered). PV matmuls
    lag 3 batches so the per-pair normalization tail never head-of-line
    blocks the PE queue. Force-pop rules guarantee every filler a matmul
    depends on is emitted earlier in the PE FIFO (else: deadlock).
  - Tail per head: DVE copy of den row 64 -> partition 0 (plain copies may
    cross quadrants; reciprocal_approx_fast works only at base 0), fast
    reciprocal, gpsimd partition_broadcast, DVE mul into aot.
  - Output projection vs Wo^T slice -> per-core partial out^T [1024, 2048];
    host sums the two head-group partials per batch and transposes.
"""

import os
from collections import deque

import numpy as np

B, T, D, H = 4, 2048, 1024, 16
HD = 64
NCORES = 8
F = 512          # per-core projection features (8 heads * 64)
P = 128          # partitions
KT = D // P      # 8 k-tiles over D
MT = F // P      # 4 m-tiles over F (also head-pairs)
NCH = 4          # token chunks
CH = T // NCH    # 512 tokens per chunk
TKT = T // P     # 16 tk tiles
NH = F // HD     # 8 local heads

_CACHE = {}


def _build():
    import concourse.bass as bass
    import concourse.tile as tile
    from concourse import bacc, mybir
    from concourse.bass import ts

    f32 = mybir.dt.float32
    bf16 = mybir.dt.bfloat16

    nc = bacc.Bacc("TRN2", target_bir_lowering=False, debug=False)

    # inputs pre-permuted host-side to [ki, ko, ...] so DMAs are contiguous
    qT = nc.dram_tensor("qT", [P, KT, T], bf16, kind="ExternalInput")
    kT = nc.dram_tensor("kT", [P, KT, T], bf16, kind="ExternalInput")
    vT = nc.dram_tensor("vT", [P, KT, T], bf16, kind="ExternalInput")
    wqT = nc.dram_tensor("wqT", [P, KT, F], bf16, kind="ExternalInput")
    wkT = nc.dram_tensor("wkT", [P, KT, F], bf16, kind="ExternalInput")
    wvT = nc.dram_tensor("wvT", [P, KT, F], bf16, kind="ExternalInput")
    woT = nc.dram_tensor("woT", [P, MT, D], bf16, kind="ExternalInput")
    bqs = nc.dram_tensor("bqs", [F], f32, kind="ExternalInput")
    bks = nc.dram_tensor("bks", [F], f32, kind="ExternalInput")
    bvs = nc.dram_tensor("bvs", [F], f32, kind="ExternalInput")
    bos = nc.dram_tensor("bos", [D], f32, kind="ExternalInput")
    outT = nc.dram_tensor("outT", [D, T], f32, kind="ExternalOutput")

    with tile.TileContext(nc) as tc:
        from contextlib import ExitStack

        with ExitStack() as ctx:
            psum = ctx.enter_context(tc.tile_pool(name="ps", bufs=1, space="PSUM"))
            const = ctx.enter_context(tc.tile_pool(name="const", bufs=1))
            persist = ctx.enter_context(tc.tile_pool(name="persist", bufs=1))
            ap = ctx.enter_context(tc.tile_pool(name="work", bufs=1))

            def raw_load(src, c, nm):
                t = ap.tile([P, KT, CH], bf16, name=nm, tag="raw", bufs=5)
                view = src[:, :, ts(c, CH)]
                # two dma_starts -> two queues -> ~2x DMA parallelism
                nc.sync.dma_start(out=t[:, 0 : KT // 2, :], in_=view[:, 0 : KT // 2, :])
                nc.sync.dma_start(out=t[:, KT // 2 :, :], in_=view[:, KT // 2 :, :])
                return t

            def w_load(src, nm):
                t = const.tile([P, KT, F], bf16, name=nm)
                view = src[:]
                nc.sync.dma_start(out=t[:, 0 : KT // 2, :], in_=view[:, 0 : KT // 2, :])
                nc.sync.dma_start(out=t[:, KT // 2 :, :], in_=view[:, KT // 2 :, :])
                return t

            # ---- constants: K/V path first so the first matmul starts early
            wk_sb = w_load(wkT, "wk_sb")
            k_raw0 = raw_load(kT, 0, "k_raw")
            wq_sb = w_load(wqT, "wq_sb")
            q_raw0 = raw_load(qT, 0, "q_raw")
            bk_sb = const.tile([P, MT], f32, name="bk_sb")
            nc.sync.dma_start(out=bk_sb, in_=bks[:].rearrange("(m p) -> p m", p=P))
            bq_sb = const.tile([P, MT], f32, name="bq_sb")
            nc.sync.dma_start(out=bq_sb, in_=bqs[:].rearrange("(m p) -> p m", p=P))
            wv_sb = w_load(wvT, "wv_sb")
            v_raw0 = raw_load(vT, 0, "v_raw")
            wo_sb = const.tile([P, MT, D], bf16, name="wo_sb")
            nc.sync.dma_start(out=wo_sb, in_=woT[:])
            bo_sb = const.tile([P, D // P], f32, name="bo_sb")
            nc.sync.dma_start(out=bo_sb, in_=bos[:].rearrange("(m p) -> p m", p=P))
            # bv broadcast across partitions (bias in [tok, feat] layout)
            bv_bc = const.tile([P, F], f32, name="bv_bc")
            bvs_ap = bvs[:]
            nc.sync.dma_start(
                out=bv_bc,
                in_=bass.AP(
                    tensor=bvs_ap.tensor, offset=bvs_ap.offset,
                    ap=[[0, P], *bvs_ap.ap],
                ),
            )
            # Touch const tiles on DVE so later fused evictions carry only a
            # single sync wait (walrus TensorScalarPtr has one wait slot).
            touch = const.tile([P, 4], f32, name="touch")
            nc.vector.tensor_copy(out=touch, in_=bq_sb)
            nc.vector.tensor_copy(out=touch, in_=bk_sb)
            nc.vector.tensor_copy(out=touch[:, 0:1], in_=bv_bc[:, 0:1])
            nc.vector.tensor_copy(out=touch[:, 0:1], in_=bo_sb[:, 0:1])

            # ---- persistent activations ----
            qpt = persist.tile([P, MT, T], bf16, name="qpt")   # qp^T  [feat, tok]
            kpt = persist.tile([P, MT, T], bf16, name="kpt")   # kp^T
            # vp4: [tok, tkt, head, HD+1]; col HD = ones (softmax denominator)
            vp4 = persist.tile([P, TKT, NH, HD + 1], bf16, name="vp4")
            nc.vector.memset(vp4[:, :, :, HD : HD + 1], 1.0)

            raws = {("k", 0): k_raw0, ("v", 0): v_raw0, ("q", 0): q_raw0}

            def kproj_tile(c, m, tag="mm", bufs=1):
                pk = psum.tile([P, CH], f32, name="pk", tag=tag, bufs=bufs)
                for k in range(KT):
                    nc.tensor.matmul(
                        pk, lhsT=wk_sb[:, k, ts(m, P)], rhs=raws[("k", c)][:, k, :],
                        start=(k == 0), stop=(k == KT - 1),
                    )
                nc.vector.tensor_scalar_add(
                    out=kpt[:, m, ts(c, CH)], in0=pk, scalar1=bk_sb[:, m : m + 1]
                )

            def vproj_tile(c, tt, tag="mm", bufs=1):
                pv_ = psum.tile([P, F], f32, name="pv_", tag=tag, bufs=bufs)
                for k in range(KT):
                    nc.tensor.matmul(
                        pv_, lhsT=raws[("v", c)][:, k, ts(tt, P)], rhs=wv_sb[:, k, :],
                        start=(k == 0), stop=(k == KT - 1),
                    )
                nc.vector.tensor_add(
                    out=vp4[:, c * NCH + tt, :, 0:HD], in0=pv_, in1=bv_bc
                )

            def qproj_tile(c, m, tag="mm", bufs=1):
                pq = psum.tile([P, CH], f32, name="pq", tag=tag, bufs=bufs)
                for k in range(KT):
                    nc.tensor.matmul(
                        pq, lhsT=wq_sb[:, k, ts(m, P)], rhs=raws[("q", c)][:, k, :],
                        start=(k == 0), stop=(k == KT - 1),
                    )
                nc.vector.tensor_scalar_add(
                    out=qpt[:, m, ts(c, CH)], in0=pq, scalar1=bq_sb[:, m : m + 1]
                )

            # ====== prologue: only what gates the first exp (K0 + Q0) ====
            for m in range(MT):
                kproj_tile(0, m, tag="pv", bufs=3)
            for m in range(MT):
                qproj_tile(0, m, tag="pv", bufs=3)

            # ============ phase 2: pipelined attention stream =============
            LAG = 3            # steady-state PV lag (batches)
            LAG_MAX = 20       # slots; < (exp bufs - 2) * 2
            pvq = deque()      # pending PV emissions: (c, p, j, exph)
            # fillers: (kind, key, fn); kinds K/V/Q keyed by chunk they fill,
            # PO keyed by the chunk whose output projection it is
            fillers = deque()
            pair_tiles = {}    # (c, p) -> (pvA, pvB)
            aot_tiles = {}     # c -> aot
            v_emitted = set()  # (chunk, tt) V-proj tiles emitted

            def pop_filler():
                _, _, fn = fillers.popleft()
                fn()

            def force(kinds, key):
                while any(k in kinds and ky <= key for k, ky, _ in fillers):
                    pop_filler()

            def emit_tail(c, p):
                pvA, pvB = pair_tiles.pop((c, p))
                aot = aot_tiles.get(c)
                if aot is None:
                    # aot slot rotation (bufs=2): all readers (out-proj of
                    # c-2) must be emitted before this alloc
                    force(("PO",), c - 2)
                    aot = ap.tile([P, MT, CH], bf16, name="aot", tag="aot", bufs=2)
                    aot_tiles[c] = aot
                denA = ap.tile([P, CH], f32, name="denA", tag="den", bufs=2)
                nc.vector.tensor_copy(out=denA[0:1, :], in_=pvA[HD : HD + 1, :])
                recA = ap.tile([P, CH], f32, name="recA", tag="rec", bufs=2)
                nc.vector.reciprocal_approx_fast(out=recA[0:1, :], in_=denA[0:1, :])
                denB = ap.tile([P, CH], f32, name="denB", tag="den", bufs=2)
                nc.vector.tensor_copy(out=denB[0:1, :], in_=pvB[HD : HD + 1, :])
                recB = ap.tile([P, CH], f32, name="recB", tag="rec", bufs=2)
                nc.vector.reciprocal_approx_fast(out=recB[0:1, :], in_=denB[0:1, :])
                bcA = ap.tile([P, CH], f32, name="bcA", tag="bc", bufs=2)
                nc.gpsimd.partition_broadcast(bcA[:, :], recA[0:1, :])
                bcB = ap.tile([P, CH], f32, name="bcB", tag="bc", bufs=2)
                nc.gpsimd.partition_broadcast(bcB[:, :], recB[0:1, :])
                nc.vector.tensor_mul(
                    out=aot[0:HD, p, :], in0=pvA[0:HD, :], in1=bcA[0:HD, :]
                )
                nc.vector.tensor_mul(
                    out=aot[HD:P, p, :], in0=pvB[0:HD, :], in1=bcB[0:HD, :]
                )

            def emit_pv(c, p, j, exph):
                # PV j needs only the single V tile (j//4, j%4)
                while (j // NCH, j % NCH) not in v_emitted and any(
                    k == "V" for k, _, _ in fillers
                ):
                    pop_filler()
                tiles = pair_tiles.get((c, p))
                if tiles is None:
                    pvA = psum.tile([P, CH], f32, name="pvA", tag="pv", bufs=3)
                    pvB = psum.tile([P, CH], f32, name="pvB", tag="pv", bufs=3)
                    tiles = pair_tiles[(c, p)] = (pvA, pvB)
                pvA, pvB = tiles
                st, sp_ = (j == 0), (j == TKT - 1)
                nc.tensor.matmul(
                    pvA[0:HD + 1, :], lhsT=vp4[:, j, 2 * p, :],
                    rhs=exph[:, 0, :], start=st, stop=sp_,
                )
                nc.tensor.matmul(
                    pvB[0:HD + 1, :], lhsT=vp4[:, j, 2 * p + 1, :],
                    rhs=exph[:, 1, :], start=st, stop=sp_,
                )
                if sp_:
                    emit_tail(c, p)

            def oproj_tile(c, m, tag="mm", bufs=1):
                po = psum.tile([P, CH], f32, name="po", tag=tag, bufs=bufs)
                for p in range(MT):
                    nc.tensor.matmul(
                        po, lhsT=wo_sb[:, p, ts(m, P)], rhs=aot_tiles[c][:, p, :],
                        start=(p == 0), stop=(p == MT - 1),
                    )
                ot = ap.tile([P, CH], f32, name="ot", tag="ot", bufs=3)
                nc.vector.tensor_scalar_add(
                    out=ot, in0=po, scalar1=bo_sb[:, m : m + 1]
                )
                nc.sync.dma_start(out=outT[ts(m, P), ts(c, CH)], in_=ot)

            def push_proj(kind, c, fn_tile, n, extra=None):
                for i in range(n):
                    def fn(i_=i, c_=c, last=(i == n - 1)):
                        fn_tile(c_, i_)
                        if kind == "V":
                            v_emitted.add((c_, i_))
                        if last and extra is not None:
                            extra()
                    fillers.append((kind, c, fn))

            # prefetch raw chunk 1; later chunks chained into the filler
            # whose emitted reads free the recycled raw buffer (bufs=3)
            raws[("k", 1)] = raw_load(kT, 1, "k_raw")
            raws[("v", 1)] = raw_load(vT, 1, "v_raw")

            def chain(kind, c):
                def fn():
                    raws[(kind, c)] = raw_load(
                        {"k": kT, "v": vT, "q": qT}[kind], c, f"{kind}_raw"
                    )
                return fn

            # deadline order: scores need K(j//4) by batch 4*chunk; PV (with
            # its elastic lag) needs V chunks ~3x later; Q1 by batch 64
            push_proj("K", 1, kproj_tile, MT, extra=chain("k", 2))
            push_proj("K", 2, kproj_tile, MT, extra=chain("k", 3))
            push_proj("K", 3, kproj_tile, MT)
            push_proj("V", 0, vproj_tile, NCH)
            push_proj("V", 1, vproj_tile, NCH, extra=chain("v", 2))
            push_proj("V", 2, vproj_tile, NCH, extra=chain("v", 3))
            push_proj("V", 3, vproj_tile, NCH, extra=chain("q", 1))

            bi = 0
            for c in range(NCH):
                if c + 1 < NCH:
                    if c >= 1:
                        chain("q", c + 1)()
                    push_proj("Q", c + 1, qproj_tile, MT)
                cs = ts(c, CH)
                sp = 1 if c == 0 else 5
                for p in range(MT):
                    for j in range(TKT):
                        force(("K",), j // NCH)  # scores need K(j//4)
                        force(("Q",), c)         # ...and Q of this chunk
                        qk = psum.tile([P, 2, CH], f32, name="qk", tag="qk", bufs=2)
                        nc.tensor.matmul(
                            qk[:, 0, :], lhsT=kpt[0:HD, p, ts(j, P)],
                            rhs=qpt[0:HD, p, cs],
                        )
                        nc.tensor.matmul(
                            qk[:, 1, :], lhsT=kpt[HD:P, p, ts(j, P)],
                            rhs=qpt[HD:P, p, cs],
                        )
                        exph = ap.tile([P, 2, CH], bf16, name="exph", tag="exp", bufs=12)
                        nc.scalar.activation(
                            out=exph, in_=qk,
                            func=mybir.ActivationFunctionType.Exp, scale=0.125,
                        )
                        pvq.append((c, p, j, exph))
                        lag_now = 1 if (c == NCH - 1 and p == MT - 1) else LAG
                        pops = 0
                        while pvq and (
                            len(pvq) > LAG_MAX
                            or (
                                len(pvq) > lag_now
                                and (pvq[0][2] // NCH, pvq[0][2] % NCH)
                                in v_emitted
                                and pops < 3
                            )
                        ):
                            emit_pv(*pvq.popleft())
                            pops += 1
                        bi += 1
                        if bi % sp == 0 and fillers:
                            pop_filler()
                if c - 1 >= 0:
                    push_proj("PO", c - 1, oproj_tile, D // P)
            while pvq:
                emit_pv(*pvq.popleft())
            while fillers:
                pop_filler()
            for m in range(D // P):
                oproj_tile(NCH - 1, m, tag="pv", bufs=3)
    nc.compile()
    return nc


def kernel(q, k, v, Wq, bq, Wk, bk, Wv, bv, Wo, bo):
    from concourse.bass_utils import run_bass_kernel_spmd

    if "nc" not in _CACHE:
        _CACHE["nc"] = _build()
    nc = _CACHE["nc"]

    q, k, v = (np.asarray(x, np.float32) for x in (q, k, v))
    Wq, Wk, Wv, Wo = (np.asarray(x, np.float32) for x in (Wq, Wk, Wv, Wo))
    bq, bk, bv, bo = (np.asarray(x, np.float32) for x in (bq, bk, bv, bo))

    import ml_dtypes

    bf = ml_dtypes.bfloat16
    in_maps = []
    for c in range(NCORES):
        b, g = c // 2, c % 2
        cols = slice(g * F, (g + 1) * F)
        def perm(xT):  # [D, N] -> [ki=128, ko=D//128, N]
            return np.ascontiguousarray(
                xT.reshape(-1, 128, xT.shape[-1]).transpose(1, 0, 2)
            )

        in_maps.append({
            "qT": perm(q[b].T.astype(bf)),
            "kT": perm(k[b].T.astype(bf)),
            "vT": perm(v[b].T.astype(bf)),
            "wqT": perm(Wq[cols, :].T.astype(bf)),
            "wkT": perm(Wk[cols, :].T.astype(bf)),
            "wvT": perm(Wv[cols, :].T.astype(bf)),
            "woT": perm(Wo[:, cols].T.astype(bf)),
            "bqs": np.ascontiguousarray(bq[cols]),
            "bks": np.ascontiguousarray(bk[cols]),
            "bvs": np.ascontiguousarray(bv[cols]),
            # bo applied once per batch (head-group 0 only)
            "bos": np.ascontiguousarray(bo if g == 0 else np.zeros_like(bo)),
        })

    _CACHE["in_maps"] = in_maps
    trace = bool(int(os.environ.get("KERNEL_TRACE", "0")))
    res = run_bass_kernel_spmd(
        nc, in_maps, core_ids=list(range(NCORES)), trace=trace
    )
    if trace and res.exec_time_ns is not None:
        print(f"HW exec time: {res.exec_time_ns} ns")
    outs = [r["outT"] for r in res.results]
    out = np.empty((B, T, D), np.float32)
    for b in range(B):
        out[b] = (outs[2 * b] + outs[2 * b + 1]).T
    return out
